# revision 40
# baseline (speedup 1.0000x reference)
"""DGCNN-style (2x DynamicEdgeConv + linear + global max pool) Trainium2 kernel.

Sharding: data-parallel over the batch dim — 8 NeuronCores x 4 graphs each.
Each core runs an identical Bass program on its 4 point clouds.

Per-graph pipeline (P=2048 points, K=20 neighbors):
  1. kNN in position space: A = ||xi-xj||^2 via one augmented fp32 matmul
     (lhsT=[-2x; 1; d2], rhs=[x; d2; 1]).  Top-20 per row on the vector
     engine: values are packed as S = (bits(A^4) & ~0x7FF) | 0x80000000 | j
     (pure bitwise; DVE arithmetic is fp32-only), which lands in the
     negative-normal fp32 range with float order == nearest-first and the low
     11 bits recovering the column index j.  Ranking on A^4 keeps the 11-bit
     quantization's near-tie flips rare.  Five max8/match_replace passes
     extract the top-20.
  2. conv1 edge MLP, decomposed: msg@W1+b1 then BN affine == a'_i + u'_j with
     a' = x@Wa+ca, u' = x@Wu.  u' columns are gathered per edge with a GPSIMD
     ap_gather (two point-halves stacked on the 128 partitions), relu'd,
     then pushed through blockdiag(W2,W2) on the tensor engine; max over the
     20 neighbors per point via tensor_reduce.
  3. kNN in 64-d feature space (same scheme, contract dim 66).
  4. conv2 decomposed: x2_i = c_i + max_j u_j, u = x1@W3b, c = x1@(W3a-W3b)+b3.
     Gather x1 columns per edge, matmul to u on the fly, segmented max, then
     one post-max c add.
  5. lin1 ([x1,x2]@Wl+bl as two accumulating matmuls) + global max over points.

Host path: the jitted PJRT executable is built ONCE and cached at module
level (run_bass_kernel_spmd would rebuild jit + re-load the NEFF on every
call).  Weights are repacked and device_put once (fingerprinted), so a
steady-state call ships only the positions (1 MB) and the donated output
zeros.  The index iota constant is generated on device (gpsimd iota).
"""

import hashlib

import numpy as np
import jax
from jax.sharding import Mesh, NamedSharding, PartitionSpec

from jax.experimental.shard_map import shard_map

import concourse.bass as bass  # noqa: F401  (engine types via nc handles)
import concourse.mybir as mybir
from concourse import bacc, bass2jax
from concourse.tile import TileContext

B, P, K = 32, 2048, 20
NCORES = 8
G = B // NCORES          # graphs per core
NT = P // 128            # 16 row-tiles per graph
MASK11 = 0xFFFFF800      # clears the 11 index bits of the fp32 distance
EH = P * K // 2          # edges per point-half (20480)
SLAB = 1920              # edges per gather/MLP slab (96 points); slab
                         # starts stay 8-byte aligned in the wrapped
                         # int16 index layout (ap_gather requirement)
MM_N = 480               # matmul moving-dim chunk (24 points)
NEG_BIG = -3.0e38

f32 = mybir.dt.float32
f32r = mybir.dt.float32  # BISECT: f32r disabled
u32 = mybir.dt.uint32
i16 = mybir.dt.int16


def _r(ap):
    """View an fp32 AP as float32r for 4x-rate PE matmuls."""
    return ap.bitcast(f32r)

W_NAMES = dict(
    wa_blk=[8, 128], wu_rep=[4, 128], w2_blk=[128, 128], b2_2=[128, 1],
    w3b2=[128, 128], w3c_aug=[65, 128], wl1_aug=[65, 128], wl2=[128, 128])
# Weights feeding float32r matmuls (4x PE rate); the rest stay fp32.
R_WEIGHT_NAMES = ('wa_blk', 'wu_rep', 'w3c_aug', 'wl1_aug',
                  'w2_blk', 'w3b2')


_cache = {}


def _build_host_tensors(W1, b1, g1, bt1, W2, b2, W3, b3, Wl, bl):
    """Data-independent repackings of the weights (host-side, tiny)."""
    W1a, W1b = W1[:3], W1[3:]
    Wa = (W1a - W1b) * g1[None, :]          # [3,64]
    ca = b1 * g1 + bt1                      # [64]
    Wu = W1b * g1[None, :]                  # [3,64]

    # a'-matmul: out [128,1024] two point-halves stacked; lhsT blockdiag.
    wa_aug = np.concatenate([Wa, ca[None, :]], 0)      # [4,64]
    wa_blk = np.zeros((8, 128), np.float32)
    wa_blk[0:4, 0:64] = wa_aug
    wa_blk[4:8, 64:128] = wa_aug

    # u'-matmul: out [128,2048] = u' replicated on both partition halves.
    wu_aug = np.concatenate([Wu, np.zeros((1, 64), np.float32)], 0)  # [4,64]
    wu_rep = np.concatenate([wu_aug, wu_aug], 1)       # [4,128]

    w2_blk = np.zeros((128, 128), np.float32)
    w2_blk[0:64, 0:64] = W2
    w2_blk[64:128, 64:128] = W2
    b2_2 = np.concatenate([b2, b2])[:, None]           # [128,1]

    W3a, W3b = W3[:64], W3[64:]
    w3b2 = np.concatenate([W3b, W3b], 0)               # [128,128] both halves
    w3c_aug = np.concatenate([W3a - W3b, b3[None, :]], 0)  # [65,128]

    wl1_aug = np.concatenate([Wl[:64], bl[None, :]], 0)    # [65,128]
    wl2 = np.ascontiguousarray(Wl[64:])                # [128,128]

    return dict(
        wa_blk=wa_blk, wu_rep=wu_rep, w2_blk=w2_blk, b2_2=b2_2,
        w3b2=w3b2, w3c_aug=w3c_aug, wl1_aug=wl1_aug, wl2=wl2,
    )


def _stt_uint(nc, out, in0, imm, in1, op0, op1, eng=None):
    """scalar_tensor_tensor with a uint32-typed immediate (bitvec ops require
    integer immediates matching src/dst dtype)."""
    eng = eng if eng is not None else nc.vector
    return eng.add_instruction(mybir.InstTensorScalarPtr(
        name=nc.get_next_instruction_name(),
        is_scalar_tensor_tensor=True, op0=op0, op1=op1,
        ins=[eng.lower_ap(in0),
             mybir.ImmediateValue(dtype=mybir.dt.uint32, value=imm),
             eng.lower_ap(in1)],
        outs=[eng.lower_ap(out)]))


def _ts_uint(nc, out, in0, imm, op0, eng=None):
    """tensor_scalar with a uint32-typed immediate."""
    eng = eng if eng is not None else nc.vector
    return eng.add_instruction(mybir.InstTensorScalarPtr(
        name=nc.get_next_instruction_name(),
        op0=op0, op1=mybir.AluOpType.bypass,
        ins=[eng.lower_ap(in0),
             mybir.ImmediateValue(dtype=mybir.dt.uint32, value=imm)],
        outs=[eng.lower_ap(out)]))



def _tt(nc, out, in0, in1, op, eng=None):
    """tensor_tensor (2-operand ALU) — gets the DVE 2x SBUF mode that
    scalar_tensor_tensor lacks."""
    eng = eng if eng is not None else nc.vector
    return eng.add_instruction(mybir.InstTensorTensor(
        name=nc.get_next_instruction_name(), op=op,
        ins=[eng.lower_ap(in0), eng.lower_ap(in1)],
        outs=[eng.lower_ap(out)]))

def _emit_knn(nc, pools, lhs, rhs, idx_dram_g):
    """Top-20 nearest neighbors for one graph; lhs/rhs are [cdim,2048] SBUF
    aug tensors (same base partition).  Writes uint16 idx to idx_dram_g."""
    pk, work = pools["pk"], pools["work"]
    iota_sb = pools["iota_sb"]
    for t in range(NT):
        # Rank on A^4: squaring twice amplifies relative gaps 4x, so the
        # 11-bit index-packing quantization flips ~4x fewer near-ties; the
        # squares also stage PSUM->SBUF for the DVE pack.  Two 1024-col
        # half-tiles (double-buffered PSUM) keep the PE streaming.
        s_t = work.tile([128, P], u32, tag="spack", name="s_t", bufs=2)
        for hf in range(2):
            a_ps = pk.tile([128, P // 2], f32, tag="pk", name="a_ps")
            for c in range(2):
                sl = slice(hf * 1024 + c * 512, hf * 1024 + (c + 1) * 512)
                nc.tensor.matmul(
                    a_ps[:, c * 512:(c + 1) * 512],
                    lhsT=lhs[:, t * 128:(t + 1) * 128],
                    rhs=rhs[:, sl],
                    start=True, stop=True)
            asq = work.tile([128, P // 2], f32, tag="asq", name="asq",
                            bufs=2)
            nc.scalar.square(asq, a_ps)
            nc.scalar.square(asq, asq)
            # pack: S = (bits(A^4) & ~0x7FF) | 0x80000000 | j -> negative
            # fp32, float-descending in distance, low 11 bits = index j.
            _stt_uint(nc, s_t[:, hf * 1024:(hf + 1) * 1024],
                      asq.bitcast(u32), MASK11,
                      iota_sb[:, hf * 1024:(hf + 1) * 1024],
                      mybir.AluOpType.bitwise_and,
                      mybir.AluOpType.bitwise_or)
        s_f = s_t.bitcast(f32)
        mbuf = work.tile([128, 24], f32, tag="mbuf", name="mbuf")
        nc.vector.max(out=mbuf[:, 0:8], in_=s_f)
        nc.vector.match_replace(out=s_f, in_to_replace=mbuf[:, 0:8],
                                in_values=s_f, imm_value=NEG_BIG)
        nc.vector.max(out=mbuf[:, 8:16], in_=s_f)
        nc.vector.match_replace(out=s_f, in_to_replace=mbuf[:, 8:16],
                                in_values=s_f, imm_value=NEG_BIG)
        nc.vector.max(out=mbuf[:, 16:24], in_=s_f)
        idx_u = work.tile([128, 24], u32, tag="idxu", name="idx_u")
        _ts_uint(nc, idx_u, mbuf.bitcast(u32), 0x7FF,
                 mybir.AluOpType.bitwise_and)
        idx16 = work.tile([128, 20], i16, tag="idx16", name="idx16", bufs=2)
        nc.vector.tensor_copy(out=idx16, in_=idx_u[:, 0:20])
        nc.sync.dma_start(out=idx_dram_g[t * 128:(t + 1) * 128, :],
                          in_=idx16)


def _load_wrapped_idx(nc, pools, idx_dram_g, name):
    """Load [2048,20] uint16 neighbor indices in the 16-wrapped per-core
    layout for indirect_copy: partitions 0-63 stream point-half A's edges,
    partitions 64-127 half B's."""
    idxw = pools["work"].tile([128, EH // 16], i16, tag="idxw", name=name)
    flat = idx_dram_g.rearrange("p k -> (p k)")
    TW = 128 * K // 16          # wrapped columns per row-tile (160)
    for grp in range(8):
        h = grp // 4
        for tl in range(NT // 2):
            src = flat[h * EH + tl * 128 * K:
                       h * EH + (tl + 1) * 128 * K].rearrange(
                           "(w q) -> q w", q=16)
            nc.sync.dma_start(
                out=idxw[grp * 16:(grp + 1) * 16, tl * TW:(tl + 1) * TW],
                in_=src)
    return idxw


def _slabs():
    out, e = [], 0
    while e < EH:
        n = min(SLAB, EH - e)
        out.append((e, n))
        e += n
    return out


def build_core_program(debug=False):
    nc = bacc.Bacc("TRN2", target_bir_lowering=False, debug=False)

    xaug_d = nc.declare_dram_parameter("xaug", [G, 4, P], f32r,
                                      isOutput=False)
    wd = {n: nc.declare_dram_parameter(n, s, (f32r if n in R_WEIGHT_NAMES else f32),
                                       isOutput=False)
          for n, s in W_NAMES.items()}
    ones_d = nc.declare_dram_parameter("ones2k", [1, P], f32r,
                                       isOutput=False)
    out_d = nc.declare_dram_parameter("out", [G, 128], f32, isOutput=True)
    if debug:
        idx_dram = nc.declare_dram_parameter(
            "idx_dbg", [G, 2, P, K], i16, isOutput=True)
        x1_dbg = nc.declare_dram_parameter(
            "x1_dbg", [G, 128, P // 2], f32, isOutput=True)
        x2_dbg = nc.declare_dram_parameter(
            "x2_dbg", [G, 128, P], f32, isOutput=True)
    else:
        idx_dram = nc.dram_tensor("idx_scratch", [G, 2, P, K], i16)

    with TileContext(nc) as tc:
        import contextlib
        ctx = contextlib.ExitStack()
        with ctx:
            const = ctx.enter_context(tc.tile_pool(name="const", bufs=1))
            persist = ctx.enter_context(tc.tile_pool(name="persist", bufs=1))
            graph = ctx.enter_context(tc.tile_pool(name="graph", bufs=2))
            work = ctx.enter_context(tc.tile_pool(name="work", bufs=2))
            pk = ctx.enter_context(
                tc.tile_pool(name="pk", bufs=2, space="PSUM"))
            psmall = ctx.enter_context(
                tc.tile_pool(name="psmall", bufs=4, space="PSUM"))

            # ---- constants ----
            # iota_sb[p, j] = 0x80000000 | j, generated on device.
            iota_sb = const.tile([128, P], u32, tag="iota_sb")
            nc.gpsimd.iota(iota_sb, pattern=[[1, P]], base=0,
                           channel_multiplier=0)
            _ts_uint(nc, iota_sb, iota_sb, 0x80000000,
                     mybir.AluOpType.bitwise_or)
            w_sb = {}
            for n, s in W_NAMES.items():
                w_sb[n] = const.tile(s, f32r if n in R_WEIGHT_NAMES
                                     else f32, tag=f"w_{n}", name=f"w_{n}")
                nc.sync.dma_start(out=w_sb[n], in_=wd[n][:, :])
            onescol = const.tile([128, 1], f32, tag="onescol")
            nc.vector.memset(onescol, 1.0)
            onescol_r = const.tile([128, 1], f32r, tag="onescol_r")
            nc.scalar.copy(onescol_r, onescol)

            pools = dict(pk=pk, psmall=psmall, work=work,
                         iota_sb=iota_sb)


            # Early-pipeline scratch, not rotated (consumed within the first
            # stage of each graph, so cross-graph reuse stalls are short).
            xaug = persist.tile([4, P], f32r, tag="xaug", name="xaug")
            xsq = persist.tile([3, P], f32, tag="xsq", name="xsq")
            xneg = persist.tile([3, P], f32r, tag="xneg", name="xneg")
            xaug2 = persist.tile([8, P // 2], f32r, tag="xaug2",
                                 name="xaug2")
            d2x1 = persist.tile([1, P], f32r, tag="d2x1", name="d2x1")
            d2x2 = persist.tile([1, P], f32r, tag="d2x2", name="d2x2")

            # Manually double-buffered kaug pair.  Row layout keeps the two
            # all-ones rows (kaug_a[64], the kNN1/kNN2/conv2/lin1 bias row,
            # and kaug_b[65]) write-once per buffer:
            #   kNN1: rhs kaug_a[64:69] = [1; d2_j; x(3)]
            #         lhs kaug_b[64:69] = [d2_i; 1; -2x(3)]
            #   kNN2: rhs kaug_a[0:66]  = [x1(64); 1; d2_j]
            #         lhs kaug_b[0:66]  = [-2x1(64); d2_i; 1]
            kaug_as, kaug_bs = [], []
            for i in range(2):
                ka = persist.tile([69, P], f32r, tag=f"kaug_a{i}",
                                  name=f"kaug_a{i}")
                kb = persist.tile([69, P], f32r, tag=f"kaug_b{i}",
                                  name=f"kaug_b{i}")
                nc.sync.dma_start(out=ka[64:65], in_=ones_d[:, :])
                nc.sync.dma_start(out=kb[65:66], in_=ones_d[:, :])
                kaug_as.append(ka)
                kaug_bs.append(kb)

            def a_knn(g):
                """xaug load, kNN1 distance+topk, conv1 pointwise preps."""
                kaug_a, kaug_b = kaug_as[g % 2], kaug_bs[g % 2]
                scr_a = graph.tile([128, P], f32r, tag="big_a",
                                   name="scr_a")
                x1rep = graph.tile([128, P], f32r, tag="big_b",
                                   name="x1rep")
                x1_2s = graph.tile([128, P // 2], f32r, tag="x1_2s",
                                   name="x1_2s")
                a2 = graph.tile([128, P // 2], f32, tag="a2", name="a2")

                # ======== positions + kNN1 aug tensors ========
                nc.sync.dma_start(out=xaug, in_=xaug_d[g])
                nc.scalar.square(xsq, xaug[0:3])
                # Engine writes must start at a 32-aligned partition; rows
                # 65-68 are filled by DMA instead (xneg stages the -2x).
                nc.scalar.activation(xneg, xaug[0:3],
                                     mybir.ActivationFunctionType.Copy,
                                     bias=0.0, scale=-2.0)
                nc.sync.dma_start(out=kaug_a[66:69], in_=xaug[0:3])
                nc.sync.dma_start(out=kaug_b[66:69], in_=xneg)
                for c in range(P // 512):
                    sl = slice(c * 512, (c + 1) * 512)
                    d2ps = psmall.tile([1, 512], f32, tag="psmall",
                                       name="d2ps")
                    nc.tensor.matmul(d2ps, lhsT=onescol[0:3],
                                     rhs=xsq[:, sl],
                                     start=True, stop=True)
                    nc.scalar.copy(d2x1[:, sl], d2ps)
                nc.sync.dma_start(out=kaug_a[65:66], in_=d2x1)
                nc.sync.dma_start(out=kaug_b[64:65], in_=d2x1)

                _emit_knn(nc, pools, kaug_b[64:69], kaug_a[64:69],
                          idx_dram[g, 0])

                # conv1 pointwise preps (PE/Act only; overlap the topk)
                urep = scr_a
                for c in range(P // 512):
                    sl = slice(c * 512, (c + 1) * 512)
                    u_ps = psmall.tile([128, 512], f32, tag="psmall",
                                       name="u_ps")
                    nc.tensor.matmul(
                        u_ps, lhsT=w_sb["wu_rep"],
                        rhs=xaug[:, sl], start=True, stop=True)
                    nc.scalar.copy(urep[:, sl], u_ps)

                nc.sync.dma_start(out=xaug2[0:4], in_=xaug_d[g][:, 0:P // 2])
                nc.sync.dma_start(out=xaug2[4:8], in_=xaug_d[g][:, P // 2:])
                for c in range(2):
                    sl = slice(c * 512, (c + 1) * 512)
                    a_ps2 = psmall.tile([128, 512], f32, tag="psmall",
                                        name="a_ps2")
                    nc.tensor.matmul(
                        a_ps2, lhsT=w_sb["wa_blk"],
                        rhs=xaug2[:, sl], start=True, stop=True)
                    nc.scalar.copy(a2[:, sl], a_ps2)
                return dict(scr_a=scr_a, x1rep=x1rep, x1_2s=x1_2s, a2=a2)

            def a_conv(g, st):
                """conv1 gather/MLP -> x1_2s (+b2), x1rep layouts."""
                scr_a, x1rep = st["scr_a"], st["x1rep"]
                x1_2s, a2 = st["x1_2s"], st["a2"]
                urep = scr_a
                idxw1 = _load_wrapped_idx(nc, pools, idx_dram[g, 0], "idxw1")
                for (e0, ne) in _slabs():
                    p0 = e0 // K
                    npts = ne // K
                    gath = work.tile([128, SLAB], f32r, tag="gath",
                                     name="gath", bufs=2)
                    nc.gpsimd.ap_gather(
                        out_ap=gath[:, 0:ne].rearrange("p (n d) -> p n d",
                                                       d=1),
                        in_ap=urep.rearrange("p (n d) -> p n d", d=1),
                        idxs_ap=idxw1[:, e0 // 16:(e0 + ne) // 16],
                        channels=128, num_elems=P, d=1, num_idxs=ne)
                    nc.vector.scalar_tensor_tensor(
                        out=gath[:, 0:ne].rearrange("p (n k) -> p n k", k=K),
                        in0=gath[:, 0:ne].rearrange("p (n k) -> p n k", k=K),
                        scalar=0.0,
                        in1=a2[:, p0:p0 + npts].unsqueeze(2).to_broadcast(
                            [128, npts, K]),
                        op0=mybir.AluOpType.bypass, op1=mybir.AluOpType.add)
                    nc.scalar.activation(gath[:, 0:ne], gath[:, 0:ne],
                                         mybir.ActivationFunctionType.Relu)
                    for c0 in range(0, ne, MM_N):
                        n = min(MM_N, ne - c0)
                        h_ps = psmall.tile([128, MM_N], f32, tag="psmall",
                                           name="h_ps")
                        nc.tensor.matmul(
                            h_ps[:, 0:n], lhsT=w_sb["w2_blk"],
                            rhs=gath[:, c0:c0 + n],
                            start=True, stop=True)
                        nc.vector.tensor_reduce(
                            out=x1_2s[:, p0 + c0 // K:p0 + (c0 + n) // K],
                            in_=h_ps[:, 0:n].rearrange(
                                "p (n k) -> p n k", k=K),
                            axis=mybir.AxisListType.X, op=mybir.AluOpType.max)
                nc.vector.tensor_scalar(
                    out=x1_2s, in0=x1_2s, scalar1=w_sb["b2_2"], scalar2=None,
                    op0=mybir.AluOpType.add)
                if debug:
                    nc.sync.dma_start(out=x1_dbg[g], in_=x1_2s)

                # ======== x1 layouts ========
                nc.sync.dma_start(out=x1rep[0:64, 0:P // 2], in_=x1_2s[0:64])
                nc.sync.dma_start(out=x1rep[0:64, P // 2:],
                                  in_=x1_2s[64:128])
                nc.sync.dma_start(out=x1rep[64:128, 0:P // 2],
                                  in_=x1_2s[0:64])
                nc.sync.dma_start(out=x1rep[64:128, P // 2:],
                                  in_=x1_2s[64:128])

            def b_knn(g, st):
                """kNN2 aug tensors + distance/topk."""
                kaug_a, kaug_b = kaug_as[g % 2], kaug_bs[g % 2]
                scr_a, x1rep = st["scr_a"], st["x1rep"]
                aug2, lhs2 = kaug_a[0:66], kaug_b[0:66]
                xsq2 = scr_a[0:64]
                nc.scalar.square(xsq2, x1rep[0:64])
                nc.scalar.copy(aug2[0:64], x1rep[0:64])
                nc.scalar.activation(lhs2[0:64], x1rep[0:64],
                                     mybir.ActivationFunctionType.Copy,
                                     bias=0.0, scale=-2.0)
                for c in range(P // 512):
                    sl = slice(c * 512, (c + 1) * 512)
                    d2ps = psmall.tile([1, 512], f32, tag="psmall",
                                       name="d2ps")
                    nc.tensor.matmul(d2ps, lhsT=onescol_r[0:64],
                                     rhs=xsq2[:, sl],
                                     start=True, stop=True)
                    nc.scalar.copy(d2x2[:, sl], d2ps)
                nc.sync.dma_start(out=aug2[65:66], in_=d2x2)
                nc.sync.dma_start(out=kaug_b[64:65], in_=d2x2)

                _emit_knn(nc, pools, lhs2, aug2, idx_dram[g, 1])

            def b_conv_p1(g, st, nslab):
                """conv2 part 1: c2, wrapped idx load, first slabs."""
                kaug_a = kaug_as[g % 2]
                scr_a, x1rep = st["scr_a"], st["x1rep"]
                aug2 = kaug_a[0:66]
                x2full = graph.tile([128, P], f32, tag="big_c",
                                    name="x2full")
                st["x2full"] = x2full
                c2 = scr_a
                for c in range(P // 512):
                    sl = slice(c * 512, (c + 1) * 512)
                    c_ps = psmall.tile([128, 512], f32, tag="psmall",
                                       name="c_ps")
                    nc.tensor.matmul(
                        c_ps, lhsT=w_sb["w3c_aug"],
                        rhs=aug2[0:65, sl],
                        start=True, stop=True)
                    nc.scalar.copy(c2[:, sl], c_ps)

                st["idxw2"] = _load_wrapped_idx(nc, pools, idx_dram[g, 1],
                                                "idxw2")
                _conv2_slabs(g, st, _slabs()[:nslab])

            def _conv2_slabs(g, st, slabs):
                x1rep, x2full, idxw2 = st["x1rep"], st["x2full"], st["idxw2"]
                for (e0, ne) in slabs:
                    p0 = e0 // K
                    gath2 = work.tile([128, SLAB], f32r, tag="gath",
                                      name="gath2", bufs=2)
                    nc.gpsimd.ap_gather(
                        out_ap=gath2[:, 0:ne].rearrange("p (n d) -> p n d",
                                                        d=1),
                        in_ap=x1rep.rearrange("p (n d) -> p n d", d=1),
                        idxs_ap=idxw2[:, e0 // 16:(e0 + ne) // 16],
                        channels=128, num_elems=P, d=1, num_idxs=ne)
                    for h in range(2):
                        pt_base = h * (P // 2) + p0
                        for c0 in range(0, ne, MM_N):
                            n = min(MM_N, ne - c0)
                            u_eps = psmall.tile([128, MM_N], f32,
                                                tag="psmall", name="u_eps")
                            nc.tensor.matmul(
                                u_eps[:, 0:n],
                                lhsT=w_sb["w3b2"][
                                    h * 64:(h + 1) * 64],
                                rhs=gath2[
                                    h * 64:(h + 1) * 64, c0:c0 + n],
                                start=True, stop=True)
                            o0 = pt_base + c0 // K
                            o1 = pt_base + (c0 + n) // K
                            nc.vector.tensor_reduce(
                                out=x2full[:, o0:o1],
                                in_=u_eps[:, 0:n].rearrange(
                                    "p (n k) -> p n k", k=K),
                                axis=mybir.AxisListType.X,
                                op=mybir.AluOpType.max)

            def b_conv_p2(g, st, nslab):
                """conv2 tail + x2 assembly + lin1 + global max -> out."""
                kaug_a = kaug_as[g % 2]
                scr_a, x2full = st["scr_a"], st["x2full"]
                aug2 = kaug_a[0:66]
                c2 = scr_a
                _conv2_slabs(g, st, _slabs()[nslab:])
                # x2 = c + max_j u_j: add c once per point, post-max.
                nc.vector.scalar_tensor_tensor(
                    out=x2full, in0=x2full, scalar=0.0, in1=c2,
                    op0=mybir.AluOpType.bypass, op1=mybir.AluOpType.add)
                if debug:
                    nc.sync.dma_start(out=x2_dbg[g], in_=x2full)

                # ======== lin1 + global max ========
                hmax = work.tile([128, 4], f32, tag="hmax", name="hmax")
                for c in range(P // 512):
                    sl = slice(c * 512, (c + 1) * 512)
                    hp_ps = psmall.tile([128, 512], f32, tag="psmall",
                                        name="hp_ps")
                    nc.tensor.matmul(
                        hp_ps, lhsT=w_sb["wl1_aug"],
                        rhs=aug2[0:65, sl],
                        start=True, stop=False)
                    nc.tensor.matmul(
                        hp_ps, lhsT=w_sb["wl2"],
                        rhs=x2full[:, sl],
                        start=False, stop=True)
                    nc.vector.tensor_reduce(
                        out=hmax[:, c:c + 1], in_=hp_ps,
                        axis=mybir.AxisListType.X, op=mybir.AluOpType.max)
                ocol = work.tile([128, 1], f32, tag="ocol", name="ocol")
                nc.vector.tensor_reduce(
                    out=ocol, in_=hmax, axis=mybir.AxisListType.X,
                    op=mybir.AluOpType.max)
                nc.sync.dma_start(out=out_d[g].unsqueeze(1), in_=ocol)

            # BISECT: sequential emission (no cross-graph interleave).
            NS1 = 4
            st = {}
            for g in range(G):
                st[g] = a_knn(g)
                a_conv(g, st[g])
                b_knn(g, st[g])
                b_conv_p1(g, st[g], NS1)
                b_conv_p2(g, st[g], NS1)
    nc.compile()
    return nc


def _make_jitted(nc, ncores):
    """Build a persistent jitted shard_map executable for `nc` (the thing
    run_bass_kernel_spmd rebuilds per call; hoisting it out makes repeat
    calls skip retrace + NEFF reload)."""
    bass2jax.install_neuronx_cc_hook()
    in_names, out_names, out_avals, out_shapes = [], [], [], []
    partition_name = (nc.partition_id_tensor.name
                      if nc.partition_id_tensor else None)
    for alloc in nc.m.functions[0].allocations:
        if not isinstance(alloc, mybir.MemoryLocationSet):
            continue
        name = alloc.memorylocations[0].name
        if alloc.kind == "ExternalInput":
            if name != partition_name:
                in_names.append(name)
        elif alloc.kind == "ExternalOutput":
            out_names.append(name)
            shape = tuple(alloc.tensor_shape)
            dtype = mybir.dt.np(alloc.dtype)
            out_avals.append(jax.core.ShapedArray(shape, dtype))
            out_shapes.append((shape, dtype))
    n_params = len(in_names)
    n_outs = len(out_avals)
    all_in_names = list(in_names) + list(out_names)
    if partition_name is not None:
        all_in_names.append(partition_name)

    def _body(*args):
        operands = list(args)
        if partition_name is not None:
            operands.append(bass2jax.partition_id_tensor())
        outs = bass2jax._bass_exec_p.bind(
            *operands, out_avals=tuple(out_avals),
            in_names=tuple(all_in_names), out_names=tuple(out_names),
            lowering_input_output_aliases=(),
            sim_require_finite=True, sim_require_nnan=True, nc=nc)
        return tuple(outs)

    devices = jax.devices()[:ncores]
    mesh = Mesh(np.asarray(devices), ("core",))
    jitted = jax.jit(
        shard_map(_body, mesh=mesh,
                  in_specs=(PartitionSpec("core"),) * (n_params + n_outs),
                  out_specs=(PartitionSpec("core"),) * n_outs,
                  check_rep=False),
        donate_argnums=tuple(range(n_params, n_params + n_outs)),
        keep_unused=True)
    return dict(jitted=jitted, in_names=in_names, out_names=out_names,
                out_shapes=out_shapes, mesh=mesh, nc=nc)


def _get_runner():
    if "runner" not in _cache:
        _cache["runner"] = _make_jitted(build_core_program(), NCORES)
    return _cache["runner"]


def kernel(pos, W1, b1, g1, bt1, W2, b2, W3, b3, Wl, bl):
    r = _get_runner()

    weights = [np.asarray(w, np.float32)
               for w in (W1, b1, g1, bt1, W2, b2, W3, b3, Wl, bl)]
    wkey = hashlib.blake2b(
        b"".join(np.ascontiguousarray(w).tobytes() for w in weights),
        digest_size=16).digest()
    if _cache.get("wkey") != wkey:
        host = _build_host_tensors(*weights)
        host["ones2k"] = np.ones((1, P), np.float32)
        shard = NamedSharding(r["mesh"], PartitionSpec("core"))
        _cache["wdev"] = {
            n: jax.device_put(
                np.ascontiguousarray(
                    np.concatenate([host[n]] * NCORES, 0), np.float32), shard)
            for n in list(W_NAMES) + ["ones2k"]}
        _cache["wkey"] = wkey

    pos = np.asarray(pos, np.float32)
    xaug = np.empty((B, 4, P), np.float32)
    xaug[:, 0:3, :] = np.transpose(pos, (0, 2, 1))
    xaug[:, 3, :] = 1.0

    args = [xaug if name == "xaug" else _cache["wdev"][name]
            for name in r["in_names"]]
    zeros = [np.zeros((NCORES * s[0], *s[1:]), d)
             for (s, d) in r["out_shapes"]]
    out_arrs = r["jitted"](*args, *zeros)
    return np.asarray(out_arrs[0]).astype(np.float32, copy=False)
